# revision 10
# baseline (speedup 1.0000x reference)
"""Multi-head latent attention (MLA) TRN2 kernel.

Sharding: batch(2) x query-sequence(4) over 8 cores. Each core:
  - computes the full KV path for its batch (kv_a, rmsnorm, kv_b, rope)
  - computes the Q path for its 512-token query chunk
  - full attention for its 512 queries x 2048 keys x 16 heads
  - o_proj for its chunk -> output slice [512, 2048]
Host assembles the 8 slices into [B, T, HID]. No collectives.

All matmul operands are bf16 (1 cycle/row on the PE like f32r, but half
the HBM traffic and no N>=256 constraint); PSUM accumulation and the
softmax statistics (sum-of-squares, rsqrt, denominators, reciprocals)
stay f32. Intermediates (q nope/rope, rotated keys, kv latent, attention
output) never leave SBUF: rope-paired layouts are built with SBUF->SBUF
scatter DMAs. Weights are host-pre-tiled as [128 part, tile, payload] so
every DMA moves >=3KB contiguous runs per partition, and each phase's
weights are prefetched during the previous phase on the ACT DGE queue
while the SP queue carries the current phase's streaming loads.

Activations are kept feature-major ([feature, token]) so weight tiles
act as lhsT directly; attention computes scores transposed
(s^T[k,q] = k^T q) so softmax needs no transposes: exp on ACT, the
denominator via an all-ones lhsT matmul, and P@V consumes the
transposed probabilities directly.
"""

import math

import numpy as np

B, T, HID = 2, 2048, 2048
NH, NKV = 16, 8
NOPE, ROPE = 128, 64
HD = NOPE + ROPE  # 192
VD = 128
KV_RANK, Q_RANK = 512, 1536
EPS = 1e-6
THETA = 10000.0
NCORES = 8
TQ = B * T // NCORES  # 512 query tokens per core
P = 128
SCALE = 1.0 / math.sqrt(HD)

# Rope rows are stored "paired": each head's rotated rope halves (32+32
# rows) are stacked into one contiguous 64-row slot at base partition
# 64*(kvh%2), so the score-matmul lhsT(k)/rhs(q) base partitions match
# (PE only allows bases {0, 32, 64}).

_CACHE = {}


def _build_nc():
    import concourse.bass as bass  # noqa: F401
    import concourse.mybir as mybir
    from concourse import bacc
    from concourse.tile import TileContext

    F32 = mybir.dt.float32
    F32R = mybir.dt.float32r
    BF16 = mybir.dt.bfloat16
    AF = mybir.ActivationFunctionType
    ALU = mybir.AluOpType

    nc = bacc.Bacc(None, target_bir_lowering=False)

    xq_d = nc.dram_tensor("xq", [P, 16, TQ], BF16, kind="ExternalInput")
    xch_d = nc.dram_tensor("xch", [P, 8, 16, 256], BF16, kind="ExternalInput")
    qa_d = nc.dram_tensor("qa_w", [P, 12, 16, P], BF16, kind="ExternalInput")
    qb_d = nc.dram_tensor("qb_w", [P, 24, 12, P], BF16, kind="ExternalInput")
    kva_d = nc.dram_tensor("kva_w", [P, 16, 1024], BF16, kind="ExternalInput")
    kvb_d = nc.dram_tensor("kvb_w", [P, 4, 2048], BF16, kind="ExternalInput")
    o_d = nc.dram_tensor("o_w", [P, 4, 16, 512], BF16, kind="ExternalInput")
    cosq_d = nc.dram_tensor("cosq", [P, TQ], BF16, kind="ExternalInput")
    sinq_d = nc.dram_tensor("sinq", [P, TQ], BF16, kind="ExternalInput")
    cosk_d = nc.dram_tensor("cosk", [P, T], BF16, kind="ExternalInput")
    sink_d = nc.dram_tensor("sink", [P, T], BF16, kind="ExternalInput")
    onesb_d = nc.dram_tensor("ones_b", [P, P], BF16, kind="ExternalInput")
    # packed f32 tables: cols 0:128 all-ones (f32r lhsT for the softmax
    # denominator matmul), 128:132 kv ln weight * sqrt(rank), 132:134 eps
    tbl_d = nc.dram_tensor("tbl", [P, 134], F32R, kind="ExternalInput")
    out_d = nc.dram_tensor("out", [TQ, HID], F32, kind="ExternalOutput")

    with TileContext(nc) as tc:
        with tc.tile_pool(name="resident", bufs=1) as res:
            kv_latN = res.tile([P, 4, T], BF16, name="kv_latN")
            qnope = res.tile([P, NH, TQ], BF16, name="qnope")
            qrope = res.tile([P, 8, TQ], BF16, name="qrope")
            kpair = res.tile([P, 4, T], BF16, name="kpair")
            attn_sb = res.tile([P, NH, TQ], BF16, name="attn_sb")
            kvb_sb = res.tile([P, 4, 2048], BF16, name="kvb_sb")

            # -- pf1: P1's inputs, prefetched during P2, freed after P1 ----
            with tc.tile_pool(name="pf1", bufs=1) as pf1:
                kva_sb = pf1.tile([P, 16, 1024], BF16, name="kva_sb")
                cosk_sb = pf1.tile([P, T], BF16, name="cosk_sb")
                sink_sb = pf1.tile([P, T], BF16, name="sink_sb")

                # ------------- P2: q path (first; no kv deps) -------------
                with (
                    tc.tile_pool(name="p2", bufs=1) as p2,
                    tc.tile_pool(name="p2w", bufs=3) as p2w,
                    tc.tile_pool(name="p2s", bufs=2) as p2s,
                    tc.tile_pool(name="p2ps", bufs=2, space="PSUM") as p2ps,
                    tc.tile_pool(name="p2ps1", bufs=1, space="PSUM") as p2ps1,
                ):
                    xq_sb = p2.tile([P, 16, TQ], BF16, name="xq_sb")
                    nc.sync.dma_start(xq_sb[:], xq_d[:, :, :])
                    ones_sb = res.tile([P, P], BF16, name="ones_sb")
                    nc.sync.dma_start(ones_sb[:], onesb_d[:, :])
                    tbl_sb = res.tile([P, 134], F32R, name="tbl_sb")
                    nc.sync.dma_start(tbl_sb[:], tbl_d[:, :])
                    cosq_sb = p2.tile([P, TQ], BF16, name="cosq_sb")
                    sinq_sb = p2.tile([P, TQ], BF16, name="sinq_sb")
                    q_lat = p2.tile([P, 12, TQ], BF16, name="q_lat")
                    rs_q = p2.tile([P, TQ], F32, name="rs_q")

                    # q_a + rmsnorm statistics (ln*rs applied after q_b:
                    # ln is folded into the q_b rows on the host, rs is a
                    # per-token scale that commutes with q_b)
                    sumsq = p2ps1.tile([P, TQ], F32, tag="qsumsq")
                    for m in range(12):
                        wt = p2w.tile([P, 16, P], BF16, tag="w")
                        nc.sync.dma_start(wt[:], qa_d[:, m, :, :])
                        ps = p2ps.tile([P, TQ], F32, tag="mm")
                        for k in range(16):
                            nc.tensor.matmul(
                                ps[:], wt[:, k, :], xq_sb[:, k, :],
                                start=(k == 0), stop=(k == 15),
                            )
                        nc.vector.tensor_copy(q_lat[:, m, :], ps[:])
                        sq = p2s.tile([P, TQ], BF16, tag="sq")
                        nc.scalar.square(sq[:], ps[:])
                        nc.tensor.matmul(
                            sumsq[:], ones_sb[:], sq[:],
                            start=(m == 0), stop=(m == 11),
                        )
                        # prefetch P1/rope inputs on the ACT DGE queue
                        if m in (2, 5, 8, 11):
                            c = (m + 1) // 3 - 1
                            nc.scalar.dma_start(
                                kva_sb[:, 4 * c : 4 * c + 4, :],
                                kva_d[:, 4 * c : 4 * c + 4, :],
                            )
                        elif m == 0:
                            nc.scalar.dma_start(cosq_sb[:], cosq_d[:, :])
                        elif m == 1:
                            nc.scalar.dma_start(sinq_sb[:], sinq_d[:, :])
                    sqt = p2s.tile([P, TQ], F32, tag="sqt")
                    nc.scalar.activation(
                        sqt[:], sumsq[:], AF.Sqrt, bias=tbl_sb[:, 133:134]
                    )
                    nc.vector.reciprocal(rs_q[:], sqt[:])

                    # q_b: nope heads to qnope, rope raw kept for rotation;
                    # the rs_q normalization rides on the PSUM->SBUF move
                    qraw1 = p2.tile([P, 4, TQ], BF16, name="qraw1")
                    qraw2 = p2.tile([P, 4, TQ], BF16, name="qraw2")
                    for m in range(24):
                        wt = p2w.tile([P, 16, P], BF16, tag="w")
                        nc.sync.dma_start(wt[:, :12, :], qb_d[:, m, :, :])
                        ps = p2ps.tile([P, TQ], F32, tag="mm")
                        for k in range(12):
                            nc.tensor.matmul(
                                ps[:], wt[:, k, :], q_lat[:, k, :],
                                start=(k == 0), stop=(k == 11),
                            )
                        if m < 16:
                            dst = qnope[:, m, :]
                        elif m < 20:
                            dst = qraw1[:, m - 16, :]
                        else:
                            dst = qraw2[:, m - 20, :]
                        nc.vector.tensor_tensor(dst, ps[:], rs_q[:], ALU.mult)
                        if m == 4:
                            nc.scalar.dma_start(cosk_sb[:], cosk_d[:, :])
                        elif m == 6:
                            nc.scalar.dma_start(sink_sb[:], sink_d[:, :])

                    # q-rope rotation then scatter to paired SBUF layout
                    cb = cosq_sb[:, None, :].to_broadcast((P, 4, TQ))
                    sb_ = sinq_sb[:, None, :].to_broadcast((P, 4, TQ))
                    qrot1 = p2.tile([P, 4, TQ], BF16, name="qrot1")
                    qrot2 = p2.tile([P, 4, TQ], BF16, name="qrot2")
                    tmp = p2.tile([P, 4, TQ], BF16, name="qrot_tmp1")
                    nc.vector.tensor_tensor(tmp[:], qraw2[:], sb_, ALU.mult)
                    nc.vector.tensor_tensor(qrot1[:], qraw1[:], cb, ALU.mult)
                    nc.vector.tensor_tensor(qrot1[:], qrot1[:], tmp[:], ALU.subtract)
                    tmp2 = p2.tile([P, 4, TQ], BF16, name="qrot_tmp2")
                    nc.vector.tensor_tensor(tmp2[:], qraw1[:], sb_, ALU.mult)
                    nc.vector.tensor_tensor(qrot2[:], qraw2[:], cb, ALU.mult)
                    nc.vector.tensor_tensor(qrot2[:], qrot2[:], tmp2[:], ALU.add)
                    # head qh -> slot 2*(qh//4)+qh%2, base 64*((qh//2)%2)
                    for qh in range(NH):
                        slot = 2 * (qh // 4) + qh % 2
                        bb = 64 * ((qh // 2) % 2)
                        src_r = (qh % 4) * 32
                        nc.scalar.dma_start(
                            qrope[bb : bb + 32, slot, :],
                            qrot1[src_r : src_r + 32, qh // 4, :],
                        )
                        nc.scalar.dma_start(
                            qrope[bb + 32 : bb + 64, slot, :],
                            qrot2[src_r : src_r + 32, qh // 4, :],
                        )

                # ------------- P1: kv_a + rmsnorm + rope ------------------
                with (
                    tc.tile_pool(name="p1", bufs=1) as p1,
                    tc.tile_pool(name="p1x", bufs=2) as p1x,
                    tc.tile_pool(name="p1s", bufs=2) as p1s,
                    tc.tile_pool(name="p1ps", bufs=2, space="PSUM") as p1ps,
                    tc.tile_pool(name="p1ps1", bufs=1, space="PSUM") as p1ps1,
                ):
                    raw1 = p1.tile([P, 2, T], BF16, name="raw1")
                    raw2 = p1.tile([P, 2, T], BF16, name="raw2")
                    for nch in range(8):
                        chsl = slice(nch * 256, (nch + 1) * 256)
                        xch = p1x.tile([P, 16, 256], BF16, tag="x")
                        nc.sync.dma_start(xch[:], xch_d[:, nch, :, :])
                        sumsq = p1ps1.tile([P, 256], F32, tag="ksumsq")
                        for m in range(8):
                            ps = p1ps.tile([P, 256], F32, tag="mm")
                            for k in range(16):
                                nc.tensor.matmul(
                                    ps[:], kva_sb[:, k, m * P : (m + 1) * P],
                                    xch[:, k, :],
                                    start=(k == 0), stop=(k == 15),
                                )
                            if m < 4:
                                nc.vector.tensor_copy(kv_latN[:, m, chsl], ps[:])
                                sq = p1s.tile([P, 256], BF16, tag="sq")
                                nc.scalar.square(sq[:], ps[:])
                                nc.tensor.matmul(
                                    sumsq[:], ones_sb[:], sq[:],
                                    start=(m == 0), stop=(m == 3),
                                )
                            elif m < 6:
                                nc.scalar.copy(raw1[:, m - 4, chsl], ps[:])
                            else:
                                nc.scalar.copy(raw2[:, m - 6, chsl], ps[:])
                        sqt = p1s.tile([P, 256], F32, tag="sqt")
                        nc.scalar.activation(
                            sqt[:], sumsq[:], AF.Sqrt, bias=tbl_sb[:, 132:133]
                        )
                        rs = p1s.tile([P, 256], F32, tag="rs")
                        nc.vector.reciprocal(rs[:], sqt[:])
                        for m in range(4):
                            nc.vector.scalar_tensor_tensor(
                                kv_latN[:, m, chsl], kv_latN[:, m, chsl],
                                tbl_sb[:, 128 + m : 129 + m], rs[:],
                                ALU.mult, ALU.mult,
                            )
                        if nch == 1:
                            # prefetch P3's kvb weights on the ACT DGE queue
                            nc.scalar.dma_start(kvb_sb[:], kvb_d[:, :, :])
                        if nch % 2 == 1:
                            # rotate the finished 512-token slab and scatter
                            sl2 = slice((nch - 1) * 256, (nch + 1) * 256)
                            ckb = cosk_sb[:, None, sl2].to_broadcast((P, 2, 512))
                            skb = sink_sb[:, None, sl2].to_broadcast((P, 2, 512))
                            rt = p1s.tile([P, 2, 512], BF16, tag="rtmp")
                            r1 = p1s.tile([P, 2, 512], BF16, tag="rot1")
                            nc.vector.tensor_tensor(rt[:], raw2[:, :, sl2], skb, ALU.mult)
                            nc.vector.tensor_tensor(r1[:], raw1[:, :, sl2], ckb, ALU.mult)
                            nc.vector.tensor_tensor(r1[:], r1[:], rt[:], ALU.subtract)
                            rt2 = p1s.tile([P, 2, 512], BF16, tag="rtmp")
                            r2 = p1s.tile([P, 2, 512], BF16, tag="rot2")
                            nc.vector.tensor_tensor(rt2[:], raw1[:, :, sl2], skb, ALU.mult)
                            nc.vector.tensor_tensor(r2[:], raw2[:, :, sl2], ckb, ALU.mult)
                            nc.vector.tensor_tensor(r2[:], r2[:], rt2[:], ALU.add)
                            # head kvh -> slot kvh//2, base 64*(kvh%2)
                            for kvh in range(NKV):
                                t_, i = kvh // 4, kvh % 4
                                bb = 64 * (kvh % 2)
                                nc.scalar.dma_start(
                                    kpair[bb : bb + 32, kvh // 2, sl2],
                                    r1[i * 32 : (i + 1) * 32, t_, :],
                                )
                                nc.scalar.dma_start(
                                    kpair[bb + 32 : bb + 64, kvh // 2, sl2],
                                    r2[i * 32 : (i + 1) * 32, t_, :],
                                )

            # ------------- P3 + P4 (pf1 SBUF freed) -----------------------
            with tc.tile_pool(name="oww", bufs=2) as oww:
                ow_tiles = {}

                def ow_load(n, eng):
                    ow = oww.tile([P, 16, 512], BF16, tag="ow")
                    eng.dma_start(ow[:], o_d[:, n, :, :])
                    ow_tiles[n] = ow

                with (
                    tc.tile_pool(name="p3", bufs=2) as p3,
                    tc.tile_pool(name="p3q", bufs=4) as p3q,
                    tc.tile_pool(name="p3p", bufs=3) as p3p,
                    tc.tile_pool(name="scps", bufs=3, space="PSUM") as scps,
                    tc.tile_pool(name="atps", bufs=2, space="PSUM") as atps,
                    tc.tile_pool(name="prps", bufs=2, space="PSUM") as prps,
                ):
                    pending = []

                    def finalize(item):
                        dsum, at, qh = item
                        dn = scps.tile([P, TQ], F32, tag="sc")
                        nc.tensor.matmul(
                            dn[:], tbl_sb[:, 0:128], dsum[:], start=True, stop=True
                        )
                        rec = p3q.tile([P, TQ], F32, tag="rec")
                        nc.vector.reciprocal(rec[:], dn[:])
                        nc.vector.tensor_tensor(
                            attn_sb[:, qh, :], at[:], rec[:], ALU.mult
                        )

                    for hp in range(4):  # kv-head pairs
                        kvh0 = 2 * hp
                        knp = p3.tile([P, 2, T], BF16, tag="knp")
                        for h2 in range(2):
                            wsl = slice((kvh0 + h2) * NOPE, (kvh0 + h2 + 1) * NOPE)
                            for n4 in range(4):
                                ksl = slice(n4 * 512, (n4 + 1) * 512)
                                ps = prps.tile([P, 512], F32, tag="pre")
                                for r in range(4):
                                    nc.tensor.matmul(
                                        ps[:], kvb_sb[:, r, wsl],
                                        kv_latN[:, r, ksl],
                                        start=(r == 0), stop=(r == 3),
                                    )
                                nc.gpsimd.tensor_copy(knp[:, h2, ksl], ps[:])
                        vp = p3.tile([P, 16, 256], BF16, tag="vp")
                        vsl = slice(NKV * NOPE + kvh0 * VD, NKV * NOPE + (kvh0 + 2) * VD)
                        for kt in range(16):
                            ps = prps.tile([P, 512], F32, tag="pre")
                            for r in range(4):
                                nc.tensor.matmul(
                                    ps[:, :256],
                                    kv_latN[:, r, kt * P : (kt + 1) * P],
                                    kvb_sb[:, r, vsl],
                                    start=(r == 0), stop=(r == 3),
                                )
                            nc.gpsimd.tensor_copy(vp[:, kt, :], ps[:, :256])

                        for j4 in range(4):
                            qh = 4 * hp + j4
                            kvh = qh // 2
                            h2 = kvh - kvh0
                            b = 64 * (kvh % 2)
                            slot = 2 * (qh // 4) + qh % 2
                            dsum = p3q.tile([P, TQ], F32R, tag="dsum")
                            at = atps.tile([P, TQ], F32, tag="at")
                            pts = {}
                            for kt in range(16):
                                sc = scps.tile([P, TQ], F32, tag="sc")
                                nc.tensor.matmul(
                                    sc[:],
                                    knp[:, h2, kt * P : (kt + 1) * P],
                                    qnope[:, qh, :],
                                    start=True, stop=False,
                                )
                                nc.tensor.matmul(
                                    sc[:],
                                    kpair[b : b + 64, kvh // 2, kt * P : (kt + 1) * P],
                                    qrope[b : b + 64, slot, :],
                                    start=False, stop=True,
                                )
                                pt = p3p.tile([P, TQ], BF16, tag="pt")
                                nc.scalar.activation(
                                    pt[:], sc[:], AF.Exp, scale=float(SCALE)
                                )
                                pts[kt] = pt
                                if kt == 0:
                                    nc.vector.tensor_copy(dsum[:], pt[:])
                                else:
                                    nc.vector.tensor_tensor(
                                        dsum[:], dsum[:], pt[:], ALU.add
                                    )
                                if kt > 0:  # PV one stage behind scores
                                    nc.tensor.matmul(
                                        at[:],
                                        vp[:, kt - 1, h2 * VD : (h2 + 1) * VD],
                                        pts[kt - 1][:],
                                        start=(kt == 1), stop=False,
                                    )
                                    del pts[kt - 1]
                            nc.tensor.matmul(
                                at[:],
                                vp[:, 15, h2 * VD : (h2 + 1) * VD],
                                pts[15][:],
                                start=False, stop=True,
                            )
                            pending.append((dsum, at, qh))
                            if len(pending) == 2:
                                finalize(pending.pop(0))
                        # prefetch P4's o_proj weights on the ACT DGE queue
                        if hp == 2:
                            ow_load(0, nc.scalar)
                        elif hp == 3:
                            ow_load(1, nc.scalar)
                    while pending:
                        finalize(pending.pop(0))

                # ------------- P4: o_proj (attn_sb resident) --------------
                with (
                    tc.tile_pool(name="p4s", bufs=2) as p4s,
                    tc.tile_pool(name="p4ps", bufs=2, space="PSUM") as p4ps,
                ):
                    for n in range(4):
                        if n not in ow_tiles:
                            ow_load(n, nc.sync)
                        ow = ow_tiles[n]
                        for mt in range(4):
                            ps = p4ps.tile([P, 512], F32, tag="o")
                            for h in range(NH):
                                nc.tensor.matmul(
                                    ps[:],
                                    attn_sb[:, h, mt * P : (mt + 1) * P],
                                    ow[:, h, :],
                                    start=(h == 0), stop=(h == 15),
                                )
                            st = p4s.tile([P, 512], F32, tag="st")
                            nc.scalar.copy(st[:], ps[:])
                            nc.sync.dma_start(
                                out_d[mt * P : (mt + 1) * P, n * 512 : (n + 1) * 512],
                                st[:],
                            )

    nc.finalize()
    return nc


def _host_prep(inputs):
    import ml_dtypes

    BF = ml_dtypes.bfloat16

    def bf(a):
        return np.ascontiguousarray(np.asarray(a, dtype=np.float32).astype(BF))

    x = np.asarray(inputs["hidden_states"], dtype=np.float32)

    qa_w = np.asarray(inputs["q_a_w"], np.float32)  # [HID, Q_RANK]
    qa_t = bf(qa_w.reshape(16, P, 12, P).transpose(1, 2, 0, 3))

    # fold the q rmsnorm weight (and the sqrt(rank) factor of the mean)
    # into the q_b rows; the per-token rsqrt is applied after q_b on-device
    lnq = (np.asarray(inputs["q_a_ln_w"], np.float64) * math.sqrt(Q_RANK)).astype(
        np.float32
    )
    qb = np.asarray(inputs["q_b_w"], np.float32) * lnq[:, None]
    qb = qb.reshape(Q_RANK, NH, HD)
    nope_cols = qb[:, :, :NOPE].reshape(Q_RANK, NH * NOPE)
    rope1 = qb[:, :, NOPE : NOPE + 32].reshape(Q_RANK, NH * 32)
    rope2 = qb[:, :, NOPE + 32 :].reshape(Q_RANK, NH * 32)
    qb_cols = np.concatenate([nope_cols, rope1, rope2], axis=1)  # [1536, 3072]
    qb_t = bf(qb_cols.reshape(12, P, 24, P).transpose(1, 2, 0, 3))

    kva = np.asarray(inputs["kv_a_w"], np.float32)
    lat = kva[:, :KV_RANK]
    krope = kva[:, KV_RANK:].reshape(HID, NKV, ROPE)
    kr1 = krope[:, :, :32].reshape(HID, NKV * 32)
    kr2 = krope[:, :, 32:].reshape(HID, NKV * 32)
    kva_cols = np.concatenate([lat, kr1, kr2], axis=1)  # [2048, 1024]
    kva_t = bf(kva_cols.reshape(16, P, 1024).transpose(1, 0, 2))

    kvb = np.asarray(inputs["kv_b_w"], np.float32).reshape(KV_RANK, NKV, NOPE + VD)
    knope_cols = kvb[:, :, :NOPE].reshape(KV_RANK, NKV * NOPE)
    v_cols = kvb[:, :, NOPE:].reshape(KV_RANK, NKV * VD)
    kvb_cols = np.concatenate([knope_cols, v_cols], axis=1)  # [512, 2048]
    kvb_t = bf(kvb_cols.reshape(4, P, 2048).transpose(1, 0, 2))

    o_w = np.asarray(inputs["o_w"], np.float32)  # [NH*VD, HID]
    o_t = bf(o_w.reshape(16, P, 4, 512).transpose(1, 2, 0, 3))

    lnkv = (
        (np.asarray(inputs["kv_a_ln_w"], np.float64) * math.sqrt(KV_RANK))
        .astype(np.float32)
        .reshape(4, P)
        .T
    )
    tbl = np.empty((P, 134), np.float32)
    tbl[:, 0:128] = 1.0
    tbl[:, 128:132] = lnkv
    tbl[:, 132] = EPS * KV_RANK
    tbl[:, 133] = EPS * Q_RANK

    inv_freq = 1.0 / (THETA ** (np.arange(0, ROPE, 2, dtype=np.float32) / ROPE))
    t = np.arange(T, dtype=np.float32)
    freqs = np.outer(t, inv_freq).astype(np.float32)
    cosk = np.tile(np.cos(freqs).T, (4, 1))  # [128, T]
    sink = np.tile(np.sin(freqs).T, (4, 1))
    cosk_b, sink_b = bf(cosk), bf(sink)
    ones_b = np.ones((P, P), BF)

    in_maps = []
    for c in range(NCORES):
        b, qc = c // 4, c % 4
        xTb = x[b].T  # [HID, T]
        qoff = qc * TQ
        xch_t = bf(xTb.reshape(16, P, 8, 256).transpose(1, 2, 0, 3))
        xq_t = bf(xTb[:, qoff : qoff + TQ].reshape(16, P, TQ).transpose(1, 0, 2))
        in_maps.append(
            {
                "xq": xq_t,
                "xch": xch_t,
                "qa_w": qa_t,
                "qb_w": qb_t,
                "kva_w": kva_t,
                "kvb_w": kvb_t,
                "o_w": o_t,
                "cosq": np.ascontiguousarray(cosk_b[:, qoff : qoff + TQ]),
                "sinq": np.ascontiguousarray(sink_b[:, qoff : qoff + TQ]),
                "cosk": cosk_b,
                "sink": sink_b,
                "ones_b": ones_b,
                "tbl": tbl,
            }
        )
    return in_maps


def get_nc():
    if "nc" not in _CACHE:
        _CACHE["nc"] = _build_nc()
    return _CACHE["nc"]


def kernel(**inputs) -> np.ndarray:
    from concourse.bass_utils import run_bass_kernel_spmd

    nc = get_nc()
    in_maps = _host_prep(inputs)
    res = run_bass_kernel_spmd(nc, in_maps, core_ids=list(range(NCORES)))
    _CACHE["last_result"] = res
    outs = [res.results[c]["out"] for c in range(NCORES)]
    full = np.stack(
        [np.concatenate([outs[b * 4 + qc] for qc in range(4)], axis=0) for b in range(B)]
    )
    return full.astype(np.float32)


# revision 14
# speedup vs baseline: 1.1126x; 1.1126x over previous
"""Multi-head latent attention (MLA) TRN2 kernel.

Sharding: batch(2) x query-sequence(4) over 8 cores. Each core:
  - computes the full KV path for its batch (kv_a, rmsnorm, kv_b, rope)
  - computes the Q path for its 512-token query chunk
  - full attention for its 512 queries x 2048 keys x 16 heads
  - o_proj for its chunk -> output slice [512, 2048]
Host assembles the 8 slices into [B, T, HID]. No collectives.

All matmul operands are bf16 (1 cycle/row on the PE like f32r, but half
the HBM traffic and no N>=256 constraint); PSUM accumulation and the
softmax statistics (sum-of-squares, rsqrt, denominators, reciprocals)
stay f32. Intermediates (q nope/rope, rotated keys, kv latent, attention
output) never leave SBUF: rope-paired layouts are built with SBUF->SBUF
scatter DMAs. Weights are host-pre-tiled as [128 part, tile, payload] so
every DMA moves >=3KB contiguous runs per partition, and each phase's
weights are prefetched during the previous phase on the ACT DGE queue
while the SP queue carries the current phase's streaming loads.

Activations are kept feature-major ([feature, token]) so weight tiles
act as lhsT directly; attention computes scores transposed
(s^T[k,q] = k^T q) so softmax needs no transposes: exp on ACT, the
denominator via an all-ones lhsT matmul, and P@V consumes the
transposed probabilities directly.
"""

import math

import numpy as np

B, T, HID = 2, 2048, 2048
NH, NKV = 16, 8
NOPE, ROPE = 128, 64
HD = NOPE + ROPE  # 192
VD = 128
KV_RANK, Q_RANK = 512, 1536
EPS = 1e-6
THETA = 10000.0
NCORES = 8
TQ = B * T // NCORES  # 512 query tokens per core
P = 128
SCALE = 1.0 / math.sqrt(HD)

# Rope rows are stored "paired": each head's rotated rope halves (32+32
# rows) are stacked into one contiguous 64-row slot at base partition
# 64*(kvh%2), so the score-matmul lhsT(k)/rhs(q) base partitions match
# (PE only allows bases {0, 32, 64}).

_CACHE = {}


def _build_nc():
    import concourse.bass as bass  # noqa: F401
    import concourse.mybir as mybir
    from concourse import bacc
    from concourse.tile import TileContext

    F32 = mybir.dt.float32
    F32R = mybir.dt.float32r
    BF16 = mybir.dt.bfloat16
    AF = mybir.ActivationFunctionType
    ALU = mybir.AluOpType

    nc = bacc.Bacc(None, target_bir_lowering=False)

    xq_d = nc.dram_tensor("xq", [P, 16, TQ], BF16, kind="ExternalInput")
    xch_d = nc.dram_tensor("xch", [P, 8, 16, 256], BF16, kind="ExternalInput")
    qa_d = nc.dram_tensor("qa_w", [P, 12, 16, P], BF16, kind="ExternalInput")
    qb_d = nc.dram_tensor("qb_w", [P, 24, 12, P], BF16, kind="ExternalInput")
    kva_d = nc.dram_tensor("kva_w", [P, 16, 1024], BF16, kind="ExternalInput")
    kvb_d = nc.dram_tensor("kvb_w", [P, 4, 2048], BF16, kind="ExternalInput")
    o_d = nc.dram_tensor("o_w", [P, 4, 16, 512], BF16, kind="ExternalInput")
    cosq_d = nc.dram_tensor("cosq", [P, TQ], BF16, kind="ExternalInput")
    sinq_d = nc.dram_tensor("sinq", [P, TQ], BF16, kind="ExternalInput")
    cosk_d = nc.dram_tensor("cosk", [P, T], BF16, kind="ExternalInput")
    sink_d = nc.dram_tensor("sink", [P, T], BF16, kind="ExternalInput")
    onesb_d = nc.dram_tensor("ones_b", [P, P], BF16, kind="ExternalInput")
    # packed f32 tables: cols 0:128 all-ones (f32r lhsT for the softmax
    # denominator matmul), 128:132 kv ln weight * sqrt(rank), 132:134 eps
    tbl_d = nc.dram_tensor("tbl", [P, 134], F32R, kind="ExternalInput")
    out_d = nc.dram_tensor("out", [TQ, HID], F32, kind="ExternalOutput")

    with TileContext(nc) as tc:
        with tc.tile_pool(name="resident", bufs=1) as res:
            kv_latN = res.tile([P, 4, T], BF16, name="kv_latN")
            qnope = res.tile([P, NH, TQ], BF16, name="qnope")
            qrope = res.tile([P, 8, TQ], BF16, name="qrope")
            kpair = res.tile([P, 4, T], BF16, name="kpair")
            attn_sb = res.tile([P, NH, TQ], BF16, name="attn_sb")
            kvb_sb = res.tile([P, 4, 2048], BF16, name="kvb_sb")

            # -- scat: rope-scatter sources, allocated at the TOP of SBUF
            # (side="right") so later phases' pools never overlap their
            # addresses and thus never wait on the background scatters.
            # -- pf1: P1's inputs, prefetched during P2, freed after P1.
            with (
                tc.tile_pool(name="scat", bufs=2, side="right") as scat,
                tc.tile_pool(name="pf1", bufs=1) as pf1,
            ):
                kva_sb = pf1.tile([P, 16, 1024], BF16, name="kva_sb")
                cosk_sb = pf1.tile([P, T], BF16, name="cosk_sb")
                sink_sb = pf1.tile([P, T], BF16, name="sink_sb")
                cosq_sb = scat.tile([P, TQ], BF16, name="cosq_sb", bufs=1)
                sinq_sb = scat.tile([P, TQ], BF16, name="sinq_sb", bufs=1)

                # ------------- P2: q path (first; no kv deps) -------------
                with (
                    tc.tile_pool(name="p2", bufs=1) as p2,
                    tc.tile_pool(name="p2w", bufs=3) as p2w,
                    tc.tile_pool(name="p2s", bufs=2) as p2s,
                    tc.tile_pool(name="p2ps", bufs=2, space="PSUM") as p2ps,
                    tc.tile_pool(name="p2ps1", bufs=1, space="PSUM") as p2ps1,
                ):
                    q_lat = p2.tile([P, 12, TQ], BF16, name="q_lat")
                    rs_q = p2.tile([P, TQ], F32, name="rs_q")

                    # q_a + rmsnorm statistics (ln*rs applied after q_b:
                    # ln is folded into the q_b rows on the host, rs is a
                    # per-token scale that commutes with q_b)
                    sumsq = p2ps1.tile([P, TQ], F32, tag="qsumsq")
                    with tc.tile_pool(name="p2xq", bufs=1) as p2xq:
                        xq_sb = p2xq.tile([P, 16, TQ], BF16, name="xq_sb")
                        wt0 = p2w.tile([P, 16, P], BF16, tag="w")
                        nc.sync.dma_start(wt0[:], qa_d[:, 0, :, :])
                        for c4 in range(4):
                            nc.sync.dma_start(
                                xq_sb[:, 4 * c4 : 4 * c4 + 4, :],
                                xq_d[:, 4 * c4 : 4 * c4 + 4, :],
                            )
                        ones_sb = res.tile([P, P], BF16, name="ones_sb")
                        nc.sync.dma_start(ones_sb[:], onesb_d[:, :])
                        tbl_sb = res.tile([P, 134], F32R, name="tbl_sb")
                        nc.sync.dma_start(tbl_sb[:], tbl_d[:, :])
                        for m in range(12):
                            if m == 0:
                                wt = wt0
                            else:
                                wt = p2w.tile([P, 16, P], BF16, tag="w")
                                nc.sync.dma_start(wt[:], qa_d[:, m, :, :])
                            ps = p2ps.tile([P, TQ], F32, tag="mm")
                            for k in range(16):
                                nc.tensor.matmul(
                                    ps[:], wt[:, k, :], xq_sb[:, k, :],
                                    start=(k == 0), stop=(k == 15),
                                )
                            nc.vector.tensor_copy(q_lat[:, m, :], ps[:])
                            sq = p2s.tile([P, TQ], BF16, tag="sq")
                            nc.scalar.square(sq[:], ps[:])
                            nc.tensor.matmul(
                                sumsq[:], ones_sb[:], sq[:],
                                start=(m == 0), stop=(m == 11),
                            )
                            # prefetch P1/rope inputs on the ACT DGE queue
                            if m in (2, 5, 8, 11):
                                c = (m + 1) // 3 - 1
                                nc.scalar.dma_start(
                                    kva_sb[:, 4 * c : 4 * c + 4, :],
                                    kva_d[:, 4 * c : 4 * c + 4, :],
                                )
                            elif m == 0:
                                nc.scalar.dma_start(cosq_sb[:], cosq_d[:, :])
                            elif m == 1:
                                nc.scalar.dma_start(sinq_sb[:], sinq_d[:, :])
                    sqt = p2s.tile([P, TQ], F32, tag="sqt", bufs=1)
                    nc.scalar.activation(
                        sqt[:], sumsq[:], AF.Sqrt, bias=tbl_sb[:, 133:134]
                    )
                    nc.vector.reciprocal(rs_q[:], sqt[:])

                    # q_b: nope heads to qnope, rope raw kept for rotation;
                    # the rs_q normalization rides on the PSUM->SBUF move.
                    # Host orders the rope halves in rounds of 8 heads:
                    # m=16,17: q1(h0-7), m=18,19: q2(h0-7),
                    # m=20,21: q1(h8-15), m=22,23: q2(h8-15).
                    qraws = {}
                    for m in range(24):
                        wt = p2w.tile([P, 16, P], BF16, tag="w")
                        nc.sync.dma_start(wt[:, :12, :], qb_d[:, m, :, :])
                        ps = p2ps.tile([P, TQ], F32, tag="mm")
                        for k in range(12):
                            nc.tensor.matmul(
                                ps[:], wt[:, k, :], q_lat[:, k, :],
                                start=(k == 0), stop=(k == 11),
                            )
                        if m < 16:
                            dst = qnope[:, m, :]
                        else:
                            j = m - 16
                            half, idx = (j // 2) % 2, j % 2
                            if idx == 0:
                                qraws[half] = p2.tile(
                                    [P, 2, TQ], BF16, tag=f"qraw{half}",
                                    bufs=2, name=f"qraw{half}",
                                )
                            dst = qraws[half][:, idx, :]
                        nc.vector.tensor_tensor(dst, ps[:], rs_q[:], ALU.mult)
                        if m == 4:
                            nc.scalar.dma_start(cosk_sb[:], cosk_d[:, :])
                        elif m == 6:
                            nc.scalar.dma_start(sink_sb[:], sink_d[:, :])
                        if m >= 16 and m % 4 == 3:
                            # rotate this round's 8 heads and scatter to the
                            # paired layout via the GPSIMD (SWDGE) queue
                            rnd = (m - 16) // 4
                            cb = cosq_sb[:, None, :].to_broadcast((P, 2, TQ))
                            sb_ = sinq_sb[:, None, :].to_broadcast((P, 2, TQ))
                            qr1, qr2 = qraws[0], qraws[1]
                            qt = p2.tile([P, 2, TQ], BF16, tag="qrtmp", bufs=2)
                            qo1 = scat.tile([P, 2, TQ], BF16, tag="qrot1")
                            qo2 = scat.tile([P, 2, TQ], BF16, tag="qrot2")
                            nc.vector.tensor_tensor(qt[:], qr2[:], sb_, ALU.mult)
                            nc.vector.tensor_tensor(qo1[:], qr1[:], cb, ALU.mult)
                            nc.vector.tensor_tensor(qo1[:], qo1[:], qt[:], ALU.subtract)
                            qt2 = p2.tile([P, 2, TQ], BF16, tag="qrtmp", bufs=2)
                            nc.vector.tensor_tensor(qt2[:], qr1[:], sb_, ALU.mult)
                            nc.vector.tensor_tensor(qo2[:], qr2[:], cb, ALU.mult)
                            nc.vector.tensor_tensor(qo2[:], qo2[:], qt2[:], ALU.add)
                            # head qh -> slot 2*(qh//4)+qh%2, base 64*((qh//2)%2)
                            for qh in range(8 * rnd, 8 * rnd + 8):
                                slot = 2 * (qh // 4) + qh % 2
                                bb = 64 * ((qh // 2) % 2)
                                src_r = (qh % 4) * 32
                                src_t = (qh % 8) // 4
                                nc.gpsimd.dma_start(
                                    qrope[bb : bb + 32, slot, :],
                                    qo1[src_r : src_r + 32, src_t, :],
                                )
                                nc.gpsimd.dma_start(
                                    qrope[bb + 32 : bb + 64, slot, :],
                                    qo2[src_r : src_r + 32, src_t, :],
                                )

                # ------------- P1: kv_a + rmsnorm + rope ------------------
                with (
                    tc.tile_pool(name="p1", bufs=1) as p1,
                    tc.tile_pool(name="p1x", bufs=3) as p1x,
                    tc.tile_pool(name="p1s", bufs=2) as p1s,
                    tc.tile_pool(name="p1ps", bufs=2, space="PSUM") as p1ps,
                    tc.tile_pool(name="p1ps1", bufs=1, space="PSUM") as p1ps1,
                ):
                    raw1 = p1.tile([P, 2, T], BF16, name="raw1")
                    raw2 = p1.tile([P, 2, T], BF16, name="raw2")
                    for nch in range(8):
                        chsl = slice(nch * 256, (nch + 1) * 256)
                        xch = p1x.tile([P, 16, 256], BF16, tag="x")
                        nc.sync.dma_start(xch[:], xch_d[:, nch, :, :])
                        sumsq = p1ps1.tile([P, 256], F32, tag="ksumsq")
                        for m in range(8):
                            ps = p1ps.tile([P, 256], F32, tag="mm")
                            for k in range(16):
                                nc.tensor.matmul(
                                    ps[:], kva_sb[:, k, m * P : (m + 1) * P],
                                    xch[:, k, :],
                                    start=(k == 0), stop=(k == 15),
                                )
                            if m < 4:
                                nc.vector.tensor_copy(kv_latN[:, m, chsl], ps[:])
                                sq = p1s.tile([P, 256], BF16, tag="sq")
                                nc.scalar.square(sq[:], ps[:])
                                nc.tensor.matmul(
                                    sumsq[:], ones_sb[:], sq[:],
                                    start=(m == 0), stop=(m == 3),
                                )
                            elif m < 6:
                                nc.scalar.copy(raw1[:, m - 4, chsl], ps[:])
                            else:
                                nc.scalar.copy(raw2[:, m - 6, chsl], ps[:])
                        sqt = p1s.tile([P, 256], F32, tag="sqt")
                        nc.scalar.activation(
                            sqt[:], sumsq[:], AF.Sqrt, bias=tbl_sb[:, 132:133]
                        )
                        rs = p1s.tile([P, 256], F32, tag="rs")
                        nc.vector.reciprocal(rs[:], sqt[:])
                        for m in range(4):
                            nc.vector.scalar_tensor_tensor(
                                kv_latN[:, m, chsl], kv_latN[:, m, chsl],
                                tbl_sb[:, 128 + m : 129 + m], rs[:],
                                ALU.mult, ALU.mult,
                            )
                        if nch == 1:
                            # prefetch P3's kvb weights on the ACT DGE queue
                            nc.scalar.dma_start(kvb_sb[:], kvb_d[:, :, :])
                        if nch % 2 == 1:
                            # rotate the finished 512-token slab and scatter
                            sl2 = slice((nch - 1) * 256, (nch + 1) * 256)
                            ckb = cosk_sb[:, None, sl2].to_broadcast((P, 2, 512))
                            skb = sink_sb[:, None, sl2].to_broadcast((P, 2, 512))
                            rt = p1s.tile([P, 2, 512], BF16, tag="rtmp")
                            r1 = scat.tile([P, 2, 512], BF16, tag="krot1")
                            nc.vector.tensor_tensor(rt[:], raw2[:, :, sl2], skb, ALU.mult)
                            nc.vector.tensor_tensor(r1[:], raw1[:, :, sl2], ckb, ALU.mult)
                            nc.vector.tensor_tensor(r1[:], r1[:], rt[:], ALU.subtract)
                            rt2 = p1s.tile([P, 2, 512], BF16, tag="rtmp")
                            r2 = scat.tile([P, 2, 512], BF16, tag="krot2")
                            nc.vector.tensor_tensor(rt2[:], raw1[:, :, sl2], skb, ALU.mult)
                            nc.vector.tensor_tensor(r2[:], raw2[:, :, sl2], ckb, ALU.mult)
                            nc.vector.tensor_tensor(r2[:], r2[:], rt2[:], ALU.add)
                            # head kvh -> slot kvh//2, base 64*(kvh%2)
                            for kvh in range(NKV):
                                t_, i = kvh // 4, kvh % 4
                                bb = 64 * (kvh % 2)
                                nc.gpsimd.dma_start(
                                    kpair[bb : bb + 32, kvh // 2, sl2],
                                    r1[i * 32 : (i + 1) * 32, t_, :],
                                )
                                nc.gpsimd.dma_start(
                                    kpair[bb + 32 : bb + 64, kvh // 2, sl2],
                                    r2[i * 32 : (i + 1) * 32, t_, :],
                                )

            # ------------- P3 + P4 (pf1 SBUF freed) -----------------------
            with tc.tile_pool(name="oww", bufs=2) as oww:
                ow_tiles = {}

                def ow_load(n, eng):
                    ow = oww.tile([P, 16, 512], BF16, tag="ow")
                    eng.dma_start(ow[:], o_d[:, n, :, :])
                    ow_tiles[n] = ow

                with (
                    tc.tile_pool(name="p3", bufs=2) as p3,
                    tc.tile_pool(name="p3q", bufs=4) as p3q,
                    tc.tile_pool(name="p3p", bufs=3) as p3p,
                    tc.tile_pool(name="scps", bufs=3, space="PSUM") as scps,
                    tc.tile_pool(name="atps", bufs=2, space="PSUM") as atps,
                    tc.tile_pool(name="prps", bufs=3, space="PSUM") as prps,
                ):
                    pending = []

                    def finalize(item):
                        dsum, at, qh = item
                        dn = scps.tile([P, TQ], F32, tag="sc")
                        nc.tensor.matmul(
                            dn[:], tbl_sb[:, 0:128], dsum[:], start=True, stop=True
                        )
                        rec = p3q.tile([P, TQ], F32, tag="rec")
                        nc.vector.reciprocal(rec[:], dn[:])
                        nc.vector.tensor_tensor(
                            attn_sb[:, qh, :], at[:], rec[:], ALU.mult
                        )

                    for hp in range(4):  # kv-head pairs
                        kvh0 = 2 * hp
                        knp = p3.tile([P, 2, T], BF16, tag="knp")
                        for h2 in range(2):
                            wsl = slice((kvh0 + h2) * NOPE, (kvh0 + h2 + 1) * NOPE)
                            for n4 in range(4):
                                ksl = slice(n4 * 512, (n4 + 1) * 512)
                                ps = prps.tile([P, 512], F32, tag="pre")
                                for r in range(4):
                                    nc.tensor.matmul(
                                        ps[:], kvb_sb[:, r, wsl],
                                        kv_latN[:, r, ksl],
                                        start=(r == 0), stop=(r == 3),
                                    )
                                nc.vector.tensor_copy(knp[:, h2, ksl], ps[:])
                        vp = p3.tile([P, 16, 256], BF16, tag="vp")
                        vsl = slice(NKV * NOPE + kvh0 * VD, NKV * NOPE + (kvh0 + 2) * VD)
                        for kt in range(16):
                            ps = prps.tile([P, 512], F32, tag="pre")
                            for r in range(4):
                                nc.tensor.matmul(
                                    ps[:, :256],
                                    kv_latN[:, r, kt * P : (kt + 1) * P],
                                    kvb_sb[:, r, vsl],
                                    start=(r == 0), stop=(r == 3),
                                )
                            nc.scalar.copy(vp[:, kt, :], ps[:, :256])

                        for j4 in range(4):
                            qh = 4 * hp + j4
                            kvh = qh // 2
                            h2 = kvh - kvh0
                            b = 64 * (kvh % 2)
                            slot = 2 * (qh // 4) + qh % 2
                            dsum = p3q.tile([P, TQ], F32R, tag="dsum")
                            at = atps.tile([P, TQ], F32, tag="at")
                            pts = {}
                            for kt in range(16):
                                sc = scps.tile([P, TQ], F32, tag="sc")
                                nc.tensor.matmul(
                                    sc[:],
                                    knp[:, h2, kt * P : (kt + 1) * P],
                                    qnope[:, qh, :],
                                    start=True, stop=False,
                                )
                                nc.tensor.matmul(
                                    sc[:],
                                    kpair[b : b + 64, kvh // 2, kt * P : (kt + 1) * P],
                                    qrope[b : b + 64, slot, :],
                                    start=False, stop=True,
                                )
                                pt = p3p.tile([P, TQ], BF16, tag="pt")
                                nc.scalar.activation(
                                    pt[:], sc[:], AF.Exp, scale=float(SCALE)
                                )
                                pts[kt] = pt
                                if kt == 0:
                                    nc.vector.tensor_copy(dsum[:], pt[:])
                                else:
                                    nc.vector.tensor_tensor(
                                        dsum[:], dsum[:], pt[:], ALU.add
                                    )
                                if kt > 0:  # PV one stage behind scores
                                    nc.tensor.matmul(
                                        at[:],
                                        vp[:, kt - 1, h2 * VD : (h2 + 1) * VD],
                                        pts[kt - 1][:],
                                        start=(kt == 1), stop=False,
                                    )
                                    del pts[kt - 1]
                            nc.tensor.matmul(
                                at[:],
                                vp[:, 15, h2 * VD : (h2 + 1) * VD],
                                pts[15][:],
                                start=False, stop=True,
                            )
                            pending.append((dsum, at, qh))
                            if len(pending) == 2:
                                finalize(pending.pop(0))
                        # prefetch P4's o_proj weights on the ACT DGE queue
                        if hp == 2:
                            ow_load(0, nc.scalar)
                        elif hp == 3:
                            ow_load(1, nc.scalar)
                    while pending:
                        finalize(pending.pop(0))

                # ------------- P4: o_proj (attn_sb resident) --------------
                with (
                    tc.tile_pool(name="p4s", bufs=2) as p4s,
                    tc.tile_pool(name="p4ps", bufs=2, space="PSUM") as p4ps,
                ):
                    for n in range(4):
                        if n not in ow_tiles:
                            ow_load(n, nc.sync)
                        ow = ow_tiles[n]
                        for mt in range(4):
                            ps = p4ps.tile([P, 512], F32, tag="o")
                            for h in range(NH):
                                nc.tensor.matmul(
                                    ps[:],
                                    attn_sb[:, h, mt * P : (mt + 1) * P],
                                    ow[:, h, :],
                                    start=(h == 0), stop=(h == 15),
                                )
                            st = p4s.tile([P, 512], F32, tag="st")
                            nc.scalar.copy(st[:], ps[:])
                            nc.sync.dma_start(
                                out_d[mt * P : (mt + 1) * P, n * 512 : (n + 1) * 512],
                                st[:],
                            )

    nc.finalize()
    return nc


def _host_prep(inputs):
    import ml_dtypes

    BF = ml_dtypes.bfloat16

    def bf(a):
        return np.ascontiguousarray(np.asarray(a, dtype=np.float32).astype(BF))

    x = np.asarray(inputs["hidden_states"], dtype=np.float32)

    qa_w = np.asarray(inputs["q_a_w"], np.float32)  # [HID, Q_RANK]
    qa_t = bf(qa_w.reshape(16, P, 12, P).transpose(1, 2, 0, 3))

    # fold the q rmsnorm weight (and the sqrt(rank) factor of the mean)
    # into the q_b rows; the per-token rsqrt is applied after q_b on-device
    lnq = (np.asarray(inputs["q_a_ln_w"], np.float64) * math.sqrt(Q_RANK)).astype(
        np.float32
    )
    qb = np.asarray(inputs["q_b_w"], np.float32) * lnq[:, None]
    qb = qb.reshape(Q_RANK, NH, HD)
    nope_cols = qb[:, :, :NOPE].reshape(Q_RANK, NH * NOPE)
    rope1 = qb[:, :, NOPE : NOPE + 32].reshape(Q_RANK, NH * 32)
    rope2 = qb[:, :, NOPE + 32 :].reshape(Q_RANK, NH * 32)
    # rope halves in rounds of 8 heads: q1(h0-7), q2(h0-7), q1(h8-15), q2(h8-15)
    qb_cols = np.concatenate(
        [nope_cols, rope1[:, :256], rope2[:, :256], rope1[:, 256:], rope2[:, 256:]],
        axis=1,
    )  # [1536, 3072]
    qb_t = bf(qb_cols.reshape(12, P, 24, P).transpose(1, 2, 0, 3))

    kva = np.asarray(inputs["kv_a_w"], np.float32)
    lat = kva[:, :KV_RANK]
    krope = kva[:, KV_RANK:].reshape(HID, NKV, ROPE)
    kr1 = krope[:, :, :32].reshape(HID, NKV * 32)
    kr2 = krope[:, :, 32:].reshape(HID, NKV * 32)
    kva_cols = np.concatenate([lat, kr1, kr2], axis=1)  # [2048, 1024]
    kva_t = bf(kva_cols.reshape(16, P, 1024).transpose(1, 0, 2))

    kvb = np.asarray(inputs["kv_b_w"], np.float32).reshape(KV_RANK, NKV, NOPE + VD)
    knope_cols = kvb[:, :, :NOPE].reshape(KV_RANK, NKV * NOPE)
    v_cols = kvb[:, :, NOPE:].reshape(KV_RANK, NKV * VD)
    kvb_cols = np.concatenate([knope_cols, v_cols], axis=1)  # [512, 2048]
    kvb_t = bf(kvb_cols.reshape(4, P, 2048).transpose(1, 0, 2))

    o_w = np.asarray(inputs["o_w"], np.float32)  # [NH*VD, HID]
    o_t = bf(o_w.reshape(16, P, 4, 512).transpose(1, 2, 0, 3))

    lnkv = (
        (np.asarray(inputs["kv_a_ln_w"], np.float64) * math.sqrt(KV_RANK))
        .astype(np.float32)
        .reshape(4, P)
        .T
    )
    tbl = np.empty((P, 134), np.float32)
    tbl[:, 0:128] = 1.0
    tbl[:, 128:132] = lnkv
    tbl[:, 132] = EPS * KV_RANK
    tbl[:, 133] = EPS * Q_RANK

    inv_freq = 1.0 / (THETA ** (np.arange(0, ROPE, 2, dtype=np.float32) / ROPE))
    t = np.arange(T, dtype=np.float32)
    freqs = np.outer(t, inv_freq).astype(np.float32)
    cosk = np.tile(np.cos(freqs).T, (4, 1))  # [128, T]
    sink = np.tile(np.sin(freqs).T, (4, 1))
    cosk_b, sink_b = bf(cosk), bf(sink)
    ones_b = np.ones((P, P), BF)

    in_maps = []
    for c in range(NCORES):
        b, qc = c // 4, c % 4
        xTb = x[b].T  # [HID, T]
        qoff = qc * TQ
        xch_t = bf(xTb.reshape(16, P, 8, 256).transpose(1, 2, 0, 3))
        xq_t = bf(xTb[:, qoff : qoff + TQ].reshape(16, P, TQ).transpose(1, 0, 2))
        in_maps.append(
            {
                "xq": xq_t,
                "xch": xch_t,
                "qa_w": qa_t,
                "qb_w": qb_t,
                "kva_w": kva_t,
                "kvb_w": kvb_t,
                "o_w": o_t,
                "cosq": np.ascontiguousarray(cosk_b[:, qoff : qoff + TQ]),
                "sinq": np.ascontiguousarray(sink_b[:, qoff : qoff + TQ]),
                "cosk": cosk_b,
                "sink": sink_b,
                "ones_b": ones_b,
                "tbl": tbl,
            }
        )
    return in_maps


def get_nc():
    if "nc" not in _CACHE:
        _CACHE["nc"] = _build_nc()
    return _CACHE["nc"]


def kernel(**inputs) -> np.ndarray:
    from concourse.bass_utils import run_bass_kernel_spmd

    nc = get_nc()
    in_maps = _host_prep(inputs)
    res = run_bass_kernel_spmd(nc, in_maps, core_ids=list(range(NCORES)))
    _CACHE["last_result"] = res
    outs = [res.results[c]["out"] for c in range(NCORES)]
    full = np.stack(
        [np.concatenate([outs[b * 4 + qc] for qc in range(4)], axis=0) for b in range(B)]
    )
    return full.astype(np.float32)
